# revision 96
# baseline (speedup 1.0000x reference)
"""Trainium2 Bass kernel for a dense CNN (conv trunk + SPP + 3 FC layers).

Sharding over 8 NeuronCores:
  - Conv trunk data-parallel over batch (8 images/core). Activations live in
    SBUF in a "strip" layout [C, H, B*W] (batch folded into width), so every
    conv matmul's moving operand is a flat contiguous slice with N in
    [256, 512] (float32r runs at full PE rate there).
  - conv1 (7x7 s2 p3) is phase-decomposed on host: 2x2 input phases turn it
    into 16 taps of a 4x4 s1 conv over phase images. 8 taps are stacked into
    the contraction dim via a host-built replicated strip; the other 8 reuse
    the same strip at a +2 column offset (second accumulating matmul).
  - conv1 packs both 460-col row-halves into one [128, 460] PSUM tile
    (second half at partition base 64) so evictions and pool1 use all 128
    partitions; pool1 runs in that packed layout and two SBUF-to-SBUF DMAs
    unpack to the [64ch, 8*37] conv2 strip.  conv2/conv3 strips are
    width-packed per layer (only valid output columns are evicted).
  - FC stage tensor-parallel: AllGather(feats, split per channel half) ->
    fc1 (512 outs/core) -> PE transpose -> AllGather(f1T, 2 halves) ->
    fc2 [64, 512] with moving weights -> PE transpose -> AllGather(f2T, 2
    halves) -> fc3 (125 outs/core) -> host-side concat of the 8 output
    slices.  The halved AllGathers let fc2/fc3 start accumulating while the
    second half is still in flight.  SPP mean division is folded into fc1
    weights on host; conv biases ride the PSUM-evict activation's bias
    port.  Latency-critical gathers ride the ACT DGE ring so they do not
    queue behind bulk weight streams on the SP ring.  Dummy matmuls paced
    by weight-prefetch DMA chunks keep the PE clock warm (HAM) across
    collective gaps.
"""

import sys

sys.path.insert(0, "/opt/trn_rl_repo")

import numpy as np
import ml_dtypes

import concourse.mybir as mybir
import concourse.tile as tile
from concourse import bacc
from concourse.bass_utils import run_bass_kernel_spmd

F32 = mybir.dt.float32
F32R = mybir.dt.float32r
BF16 = mybir.dt.bfloat16
F8E4 = mybir.dt.float8e4
F8E5 = mybir.dt.float8e5
DR = mybir.MatmulPerfMode.DoubleRowSwInterleave
E4NP = ml_dtypes.float8_e4m3
E5NP = ml_dtypes.float8_e5m2
RELU = mybir.ActivationFunctionType.Relu
COPY = mybir.ActivationFunctionType.Copy
MAX = mybir.AluOpType.max
ADD = mybir.AluOpType.add
AXX = mybir.AxisListType.X

DEBUG = False
N_CORES = 8
B = 64
BL = B // N_CORES
W1S = 922        # conv1 strip row width (8*115 + 2 pad)
C1BAND = 9       # conv1 band rows (multiple of 3)
SPP_LEVELS = (6, 3, 2, 1)


def _bins(L, H=10):
    return [((i * H) // L, -((-(i + 1) * H) // L)) for i in range(L)]


def _spp_bins():
    bins = []
    for L in SPP_LEVELS:
        bd = _bins(L)
        for i0, i1 in bd:
            for j0, j1 in bd:
                bins.append((i0, i1, j0, j1))
    return bins


# ----------------------------------------------------------------------------
# device program
# ----------------------------------------------------------------------------

def build_program():
    nc = bacc.Bacc(None, target_bir_lowering=False)

    def din(name, shape, dt):
        return nc.dram_tensor(name, list(shape), dt, kind="ExternalInput")

    c1rep = din("c1rep", [96, 111, W1S], BF16)
    w1g = din("w1g", [2, 96, 64], BF16)
    b1 = din("b1", [128, 1], F32)
    w2s = [din(f"w2_{i}", [2, 2, 64 if i == 0 else 128, 128], F32R) for i in range(4)]
    b2s = [din(f"b2_{i}", [128, 1], F32) for i in range(4)]
    w3s = [din(f"w3_{i}", [2, 2, 128 if i == 0 else 256, 256], BF16) for i in range(6)]
    b3s = [din(f"b3_{i}", [128, 2], F32) for i in range(6)]
    w1c = din("w1c", [13, 128, 8, 512], BF16)
    b1c = din("b1c", [1, 512], BF16)
    w2c = din("w2c", [128, 32, 512], BF16)
    b2c = din("b2c", [1, 512], BF16)
    w3T = din("w3T", [128, 32, 125], BF16)
    b3f = din("b3f", [1, 125], BF16)
    identI = din("ident", [64, 64], BF16)

    out = nc.dram_tensor("out", [64, 125], F32, kind="ExternalOutput")
    if DEBUG:
        dbg_feats = nc.dram_tensor("dbg_feats", [128, 2, BL, 50], BF16,
                                   kind="ExternalOutput")
        dbg_f1 = nc.dram_tensor("dbg_f1", [64, 512], BF16, kind="ExternalOutput")
        dbg_f2T = nc.dram_tensor("dbg_f2T", [128, 4, B], BF16,
                                 kind="ExternalOutput")
        dbg_h5 = nc.dram_tensor("dbg_h5", [128, 2, 16, 128], BF16,
                                kind="ExternalOutput")

    ag_srcs = [nc.dram_tensor(f"ag{c}_src", [128, BL, 50], BF16) for c in range(2)]
    ag_dsts = [nc.dram_tensor(f"ag{c}_dst", [N_CORES, 128, BL, 50], BF16,
                              addr_space="Shared") for c in range(2)]
    agf1_srcs = [nc.dram_tensor(f"agf1_src{h}", [128, 2, B], BF16)
                 for h in range(2)]
    agf1_dsts = [nc.dram_tensor(f"agf1_dst{h}", [N_CORES, 128, 2, B], BF16,
                                addr_space="Shared") for h in range(2)]
    agf2_srcs = [nc.dram_tensor(f"agf2_src{h}", [128, 2, B], BF16)
                 for h in range(2)]
    agf2_dsts = [nc.dram_tensor(f"agf2_dst{h}", [N_CORES, 128, 2, B], BF16,
                                addr_space="Shared") for h in range(2)]
    warm_src = nc.dram_tensor("warm_src", [1, 16], F32)
    warm_dst = nc.dram_tensor("warm_dst", [N_CORES, 16], F32, addr_space="Shared")

    tc_cm = tile.TileContext(nc)
    tc = tc_cm.__enter__()

    const_cm = tc.tile_pool(name="const", bufs=1); const = const_cm.__enter__()
    w1_cm = tc.tile_pool(name="w1pool", bufs=5); w1pool = w1_cm.__enter__()
    psum_holder = {}

    def ps(name):
        return psum_holder["pool"].tile([128, 512], F32, name=name, tag="ps")

    # --- small resident constants -------------------------------------------
    w1sb = const.tile([96, 2, 64], BF16, name="w1sb")
    nc.sync.dma_start(w1sb[:], w1g[:].transpose((1, 0, 2)))
    b1sb = const.tile([128, 1], F32, name="b1sb")
    nc.sync.dma_start(b1sb[:], b1[:])
    # ========================================================================
    # conv1 + pool1
    # ========================================================================
    mid_cm = tc.tile_pool(name="midpool", bufs=1); midpool = mid_cm.__enter__()
    p1_cm = tc.tile_pool(name="p1pool", bufs=1); p1pool = p1_cm.__enter__()
    pooled1 = p1pool.tile([64, 38, 296], F32R, name="pooled1")
    nc.vector.memset(pooled1[:, 37, :].bitcast(F32), 0.0)

    psum_cm = tc.tile_pool(name="psum", bufs=4, space="PSUM")
    psum_holder["pool"] = psum = psum_cm.__enter__()
    band_cm = tc.tile_pool(name="band", bufs=2); band_pool = band_cm.__enter__()
    # packed pool1 accumulator: partitions = (col-half, 64ch); imgs 0-3 in
    # lower 64 partitions, imgs 4-7 in upper.  Lives in the band pool (dead
    # after the conv1 loop) so its SBUF returns before conv2's a2pool opens.
    pooled1p = band_pool.tile([128, 37, 148], F32R, name="pooled1p",
                              tag="p1p", bufs=1)
    r0 = 0
    while r0 < 111:
        nr = min(C1BAND, 111 - r0)
        rep = band_pool.tile([96, C1BAND, W1S], BF16, name="rep", tag="rep",
                             bufs=4)
        nc.sync.dma_start(rep[:, :nr, :], c1rep[:, r0:r0 + nr, :])
        c1o = band_pool.tile([128, C1BAND, 460], BF16, name="c1o", tag="c1o")
        rep_flat = rep[:].rearrange("k h w -> k (h w)")
        for r in range(nr):
            p = ps("p_c1")
            for hf, u0 in ((0, 0), (1, 460)):
                for wg in range(2):
                    base = r * W1S + u0 + 2 * wg
                    nc.tensor.matmul(p[64 * hf:64 * (hf + 1), :460],
                                     w1sb[:, wg, :],
                                     rep_flat[:, base:base + 460],
                                     start=(wg == 0), stop=(wg == 1))
            nc.scalar.activation(c1o[:, r, :], p[:, :460], RELU, bias=b1sb[:])
        pr0, prn = r0 // 3, nr // 3
        t1 = band_pool.tile([128, C1BAND, 148], BF16, name="t1", tag="t1",
                            bufs=1)
        cv = c1o[:, :nr, :].rearrange("c h (b w) -> c h b w", w=115)
        t1v = t1[:, :nr, :].rearrange("c h (b w) -> c h b w", w=37)
        nc.vector.tensor_tensor(t1v, cv[:, :, :, 0:111:3], cv[:, :, :, 1:112:3], MAX)
        nc.vector.tensor_tensor(t1v, t1v, cv[:, :, :, 2:113:3], MAX)
        pv = pooled1p[:, pr0:pr0 + prn, :]
        nc.vector.tensor_tensor(pv, t1[:, 0:3 * prn:3, :], t1[:, 1:3 * prn:3, :], MAX)
        nc.vector.tensor_tensor(pv, pv, t1[:, 2:3 * prn:3, :], MAX)
        # unpack col-halves back to [64ch, rows, 8*37] via sbuf-to-sbuf DMA
        # (gpsimd SWDGE: latency-tolerant, keeps dep-waiting DMAs off both
        # HWDGE rings so band loads never stall behind this band's pooling)
        nc.gpsimd.dma_start(pooled1[:, pr0:pr0 + prn, 0:148],
                            pooled1p[0:64, pr0:pr0 + prn, :])
        nc.gpsimd.dma_start(pooled1[:, pr0:pr0 + prn, 148:296],
                            pooled1p[64:128, pr0:pr0 + prn, :])
        r0 += nr
        if r0 == 4 * C1BAND:
            # emit the late-phase constant loads once band fill has headroom
            w2sb, b2sb = [], []
            for i in range(4):
                cin = 64 if i == 0 else 128
                t = const.tile([cin, 2, 2, 128], F32R, name=f"w2sb{i}")
                nc.scalar.dma_start(t[:], w2s[i][:].transpose((2, 0, 1, 3)))
                w2sb.append(t)
                tb = const.tile([128, 1], F32, name=f"b2sb{i}")
                nc.scalar.dma_start(tb[:], b2s[i][:])
                b2sb.append(tb)
            b3sb = []
            for i in range(6):
                tb = const.tile([128, 2], F32, name=f"b3sb{i}")
                nc.scalar.dma_start(tb[:], b3s[i][:])
                b3sb.append(tb)
            ones_bf = const.tile([1, 64], BF16, name="ones_bf")
            nc.vector.memset(ones_bf[:], 1.0)
            b1csb = const.tile([1, 512], BF16, name="b1csb")
            nc.scalar.dma_start(b1csb[:], b1c[:])
            b2csb = const.tile([1, 512], BF16, name="b2csb")
            nc.scalar.dma_start(b2csb[:], b2c[:])
            b3fsb = const.tile([1, 125], BF16, name="b3fsb")
            nc.scalar.dma_start(b3fsb[:], b3f[:])
            identsb = const.tile([64, 64], BF16, name="identsb")
            nc.scalar.dma_start(identsb[:], identI[:])
            # warm up the collectives firmware with a tiny AllGather
            warm_sb = const.tile([1, 16], F32, name="warm_sb")
            nc.vector.memset(warm_sb[:], 0.0)
            nc.scalar.dma_start(warm_src[:], warm_sb[:])
            nc.gpsimd.collective_compute(
                "AllGather", mybir.AluOpType.bypass,
                replica_groups=[list(range(N_CORES))],
                ins=[warm_src[:].opt()], outs=[warm_dst[:].opt()])
    band_cm.__exit__(None, None, None)


    # ========================================================================
    # conv2 block (4 layers), strip width 37/img
    # ========================================================================
    a2_cm = tc.tile_pool(name="a2pool", bufs=2); a2pool = a2_cm.__enter__()
    cur = pooled1
    for li in range(4):
        w_in = 37 - li
        w_out = w_in - 1
        hout = 36 - li
        w = w2sb[li]
        dst = a2pool.tile([128, hout + 1, 8 * w_out], F32R, name=f"a2_{li}",
                          tag="a2")
        nc.vector.memset(dst[:, hout, :].bitcast(F32), 0.0)
        cur_flat = cur[:].rearrange("c h w -> c (h w)")
        for r in range(hout):
            p = ps("p_c2")
            k = 0
            for di in range(2):
                for dj in range(2):
                    base = (r + di) * 8 * w_in + dj
                    nc.tensor.matmul(p[:, :8 * w_in], w[:, di, dj, :],
                                     cur_flat[:, base:base + 8 * w_in],
                                     start=(k == 0), stop=(k == 3))
                    k += 1
            pv = p[:, :8 * w_in].rearrange("c (b w) -> c b w", w=w_in)
            dv = dst[:, r, :].rearrange("c (b w) -> c b w", w=w_out)
            nc.scalar.activation(dv, pv[:, :, :w_out], RELU,
                                 bias=b2sb[li][:])
        cur = dst

    # pool2: 2x2 s2 (33 valid rows, 33 cols/img) -> [128, 16, 8*16] bf16
    pooled2 = midpool.tile([128, 17, 128], BF16, name="pooled2")
    nc.vector.memset(pooled2[:, 16, :], 0.0)
    t2 = a2pool.tile([128, 33, 128], BF16, name="t2", tag="t2", bufs=1)
    va = cur[:, :33, :].bitcast(F32).rearrange("c h (b w) -> c h b w", w=33)
    t2v = t2[:].rearrange("c h (b w) -> c h b w", w=16)
    for rr in range(0, 33, 9):
        rn = min(9, 33 - rr)
        nc.vector.tensor_tensor(t2v[:, rr:rr + rn], va[:, rr:rr + rn, :, 0:32:2],
                                va[:, rr:rr + rn, :, 1:33:2], MAX)
    for rr in range(0, 16, 4):
        nc.vector.tensor_tensor(pooled2[:, rr:rr + 4, :],
                                t2[:, 2 * rr:2 * rr + 8:2, :],
                                t2[:, 2 * rr + 1:2 * rr + 9:2, :], MAX)
    a2_cm.__exit__(None, None, None)
    p1_cm.__exit__(None, None, None)

    # fc pools enter first so the conv3-scoped pools above them can be
    # released in LIFO order after SPP
    fc_cm = tc.tile_pool(name="fcpool", bufs=1); fcpool = fc_cm.__enter__()
    w2p_cm = tc.tile_pool(name="w2pool", bufs=1); w2pool = w2p_cm.__enter__()
    w3p_cm = tc.tile_pool(name="w3fpool", bufs=1); w3fpool = w3p_cm.__enter__()

    # conv3 weight pool + first two layers' weights BEFORE the fc1 weight
    # stream so conv3 layer 0 isn't stuck behind 13 MB of w1c DMA traffic.
    w3_cm = tc.tile_pool(name="w3pool", bufs=3); w3pool = w3_cm.__enter__()

    def load_w3(li):
        nkt = 1 if li == 0 else 2
        wsb = w3pool.tile([128, nkt, 2, 2, 256], BF16, name=f"w3sb{li}", tag="w3s")
        for kt in range(nkt):
            nc.sync.dma_start(
                wsb[:, kt, :, :, :],
                w3s[li][:, :, kt * 128:(kt + 1) * 128, :].transpose((2, 0, 1, 3)))
        return wsb

    w3sb_tiles = {0: load_w3(0), 1: load_w3(1)}

    # fc1 weight tiles, 8 K-tiles per DMA (stream during conv3 + fc1)
    w1tiles = []
    for g in range(13):
        nkt = 8 if g < 12 else 4
        wt = w1pool.tile([128, 8, 512], BF16, name="w1t", tag="w1t")
        nc.sync.dma_start(wt[:, :nkt, :], w1c[g, :, :nkt, :])
        w1tiles.append(wt)

    # ========================================================================
    # conv3 block (6 layers), strip width 16/img, bf16
    # ========================================================================
    fe_cm = tc.tile_pool(name="fepool", bufs=1); fepool = fe_cm.__enter__()
    a3_cm = tc.tile_pool(name="a3pool", bufs=2); a3pool = a3_cm.__enter__()

    a3prev = None
    for li in range(6):
        nkt = 1 if li == 0 else 2
        hin = 16 - li
        hout = hin - 1
        w_in = 16 - li
        w_out = w_in - 1
        wsb = w3sb_tiles[li]
        dst = a3pool.tile([128, 2, hout + 1, 8 * w_out], BF16,
                          name=f"a3_{li}", tag="a3")
        nc.vector.memset(dst[:, :, hout, :], 0.0)
        if li == 0:
            src_flat = [pooled2[:].rearrange("c h w -> c (h w)")]
        else:
            src_flat = [a3prev[:, kt, :, :].rearrange("c h w -> c (h w)")
                        for kt in range(2)]
        rcs = []
        r = 0
        while r < hout:
            rcs.append((r, min(3, hout - r)))
            r += rcs[-1][1]
        if li == 5:
            loop = [(mt, r, nr) for mt in range(2) for (r, nr) in rcs]
        else:
            loop = [(mt, r, nr) for (r, nr) in rcs for mt in range(2)]
        for (mt, r, nr) in loop:
            p = ps("p_c3")
            k = 0
            nmm = 4 * nkt
            for kt in range(nkt):
                for di in range(2):
                    for dj in range(2):
                        base = (r + di) * 8 * w_in + dj
                        nc.tensor.matmul(
                            p[:, :nr * 8 * w_in],
                            wsb[:, kt, di, dj, mt * 128:(mt + 1) * 128],
                            src_flat[kt][:, base:base + nr * 8 * w_in],
                            start=(k == 0), stop=(k == nmm - 1))
                        k += 1
            pv = p[:, :nr * 8 * w_in].rearrange("c (h b w) -> c h b w", w=w_in,
                                                h=nr)
            dv = dst[:, mt, r:r + nr, :].rearrange("c h (b w) -> c h b w",
                                                   w=w_out)
            nc.scalar.activation(dv, pv[:, :, :, :w_out], RELU,
                                 bias=b3sb[li][:, mt:mt + 1])
        # prefetch next-next layer weights mid-layer
        if li + 2 <= 5:
            w3sb_tiles[li + 2] = load_w3(li + 2)
        a3prev = dst
    h5 = a3prev

    # ========================================================================
    # SPP (hierarchical bin sums; mean folded into fc1 weights) -> feats bf16
    # feats layout [128ch, 50 bin, 8 img] per ct so fc1's lhsT slices are
    # contiguous-ish.  ct=0 SPP+AllGather issued right after conv3 layer 5's
    # mt=0 half so the collective overlaps the mt=1 matmuls.
    # ========================================================================
    rb_base = {}        # 12 row bins, level-major
    _n = 0
    for L in SPP_LEVELS:
        rb_base[L] = _n
        _n += L

    feats = fepool.tile([128, 2, BL, 50], BF16, name="feats")
    rs = fepool.tile([128, 2, 12, 80], BF16, name="rs")
    cr = fepool.tile([128, 2, 12, 12, BL], BF16, name="cr")

    def spp_ct(ct):
        row = lambda r: h5[:, ct, r, :]
        rb = lambda i: rs[:, ct, i, :]
        # row bins: L6 direct, higher levels reuse lower sums
        for i, (i0, i1) in enumerate(_bins(6)):
            nc.vector.tensor_tensor(rb(i), row(i0), row(i0 + 1), ADD)
            for r in range(i0 + 2, i1):
                nc.vector.tensor_tensor(rb(i), rb(i), row(r), ADD)
        # L3: {0-3}=L6b0+r2+r3 ; {3-6}=L6b3{5,6}+r3+r4 ; {6-9}=L6b5{8,9}+r6+r7
        for i, (src, extra) in enumerate([(0, (2, 3)), (3, (3, 4)), (5, (6, 7))]):
            nc.vector.tensor_tensor(rb(6 + i), rb(src), row(extra[0]), ADD)
            nc.vector.tensor_tensor(rb(6 + i), rb(6 + i), row(extra[1]), ADD)
        # L2: {0-4}=L3b0+r4 ; {5-9}=L3b2+r5
        nc.vector.tensor_tensor(rb(9), rb(6), row(4), ADD)
        nc.vector.tensor_tensor(rb(10), rb(8), row(5), ADD)
        # L1
        nc.vector.tensor_tensor(rb(11), rb(9), rb(10), ADD)
        # column bins: one reduce per (level, j) over all 12 row bins
        rsw = rs[:, ct].rearrange("c rb (b w) -> c rb b w", w=10)
        crv = cr[:, ct]
        for L in SPP_LEVELS:
            cb0 = rb_base[L]
            for j, (j0, j1) in enumerate(_bins(L)):
                nc.vector.tensor_reduce(crv[:, cb0 + j, :, :],
                                        rsw[:, :, :, j0:j1], AXX, ADD)
        kbase = 0
        for L in SPP_LEVELS:
            rb0 = rb_base[L]
            dstv = feats[:, ct, :, kbase:kbase + L * L].rearrange(
                "c b (i j) -> c b i j", j=L)
            nc.vector.tensor_copy(
                dstv,
                crv[:, rb0:rb0 + L, rb0:rb0 + L, :].rearrange("c j i b -> c b i j"))
            kbase += L * L
        nc.sync.dma_start(ag_srcs[ct][:], feats[:, ct])
        nc.gpsimd.collective_compute(
            "AllGather", mybir.AluOpType.bypass,
            replica_groups=[list(range(N_CORES))],
            ins=[ag_srcs[ct][:].opt()], outs=[ag_dsts[ct][:].opt()])

    # Emission after the full conv3 loop is fine: spp ct=0's data deps only
    # reference the mt=0 half, so its DVE ops + AllGather #1 overlap the
    # mt=1 matmuls at execution time.
    with nc.allow_low_precision(reason="SPP small-window sums"):
        spp_ct(0)
        spp_ct(1)
    if DEBUG:
        nc.sync.dma_start(dbg_feats[:], feats[:])
        nc.sync.dma_start(dbg_h5[:], h5[:])

    a3_cm.__exit__(None, None, None)
    fe_cm.__exit__(None, None, None)
    w3_cm.__exit__(None, None, None)

    # ========================================================================
    # fc1: [64, 512] = feats_full.T @ w1c (+bias), relu.  ct=0 matmuls only
    # need the first AllGather, so they overlap AllGather #2.
    # ========================================================================
    # fc2/fc3 weight prefetch first (no deps -> streams during conv3 tail /
    # the feats AllGathers)
    w2sb2 = w2pool.tile([128, 32, 512], BF16, name="w2sb2")
    for i in range(8):
        nc.sync.dma_start(
            w2sb2[:, 4 * i:4 * i + 4, :],
            w2c[:, 4 * i:4 * i + 4, :])
    w3fsb = w3fpool.tile([128, 32, 125], BF16, name="w3fsb")
    nc.sync.dma_start(w3fsb[:], w3T[:])
    scratch = w2pool.tile([128, 1, 512], BF16, name="scratch")

    def warm_mm():
        # shares the pf2 tag/shape: only used while no fc2 chain is live
        pw = psum.tile([128, 4, B], F32, name="pf2", tag="pf2", bufs=1)
        wv = pw[:64].rearrange("p a b -> p (a b)")
        sv = scratch[:].rearrange("p a b -> p (a b)")
        for j in range(6):
            nc.tensor.matmul(wv, scratch[:, 0, 0:64],
                             sv[:, 256 * (j % 2):256 * (j % 2) + 256],
                             start=True, stop=True)

    def pacer(n, base):
        # serialized scratch re-DMAs, each feeding dummy matmul bursts: keeps
        # the PE clock (HAM) warm through a collective gap at ~1.5us spacing
        for i in range(n):
            nc.sync.dma_start(
                scratch[:],
                w2c[:, (base + i) % 8:(base + i) % 8 + 1, :])
            warm_mm()

    pf1 = psum.tile([64, 512], F32, name="pf1", tag="pf1", bufs=1)
    pacer(8, 0)
    for ci in range(2):
        featg = fcpool.tile([128, N_CORES, BL, 50], BF16, name=f"featg{ci}")
        for c8 in range(N_CORES):
            nc.scalar.dma_start(featg[:, c8], ag_dsts[ci][c8])
        for k in range(50):
            kt = 50 * ci + k
            nc.tensor.matmul(pf1[:], featg[:, :, :, k],
                             w1tiles[kt // 8][:, kt % 8, :],
                             start=(kt == 0), stop=False)
    nc.tensor.matmul(pf1[:], ones_bf[:], b1csb[:], start=False, stop=True)
    f1 = fcpool.tile([64, 512], BF16, name="f1")
    nc.scalar.activation(f1[:], pf1[:], RELU)
    if DEBUG:
        nc.sync.dma_start(dbg_f1[:], f1[:])

    # transpose f1 -> f1T [128, 4, 64] via PE transpose; AllGather per half so
    # fc2 starts accumulating while the second half is in flight
    f1T = fcpool.tile([128, 4, B], BF16, name="f1T")
    for h in range(2):
        pt = psum.tile([128, 2, 64], BF16, name="pt", tag="pt", bufs=1)
        for i, t in enumerate((2 * h, 2 * h + 1)):
            nc.tensor.transpose(pt[:, i, :], f1[:, 128 * t:128 * (t + 1)],
                                identsb[:])
        nc.scalar.activation(f1T[:, 2 * h:2 * h + 2, :].rearrange(
            "p a b -> p (a b)"), pt[:].rearrange("p a b -> p (a b)"), COPY)
        nc.scalar.dma_start(agf1_srcs[h][:], f1T[:, 2 * h:2 * h + 2, :])
        nc.gpsimd.collective_compute(
            "AllGather", mybir.AluOpType.bypass,
            replica_groups=[list(range(N_CORES))],
            ins=[agf1_srcs[h][:].opt()], outs=[agf1_dsts[h][:].opt()])

    pacer(4, 0)

    # ========================================================================
    # fc2: f2[64, 512] = f1_full @ w2 slice (+bias, relu), then PE transpose
    # ========================================================================
    f2 = fcpool.tile([64, 512], BF16, name="f2")
    pf2b = psum.tile([64, 512], F32, name="pf2b", tag="pf1", bufs=1)
    kt2 = 0
    for h in range(2):
        f1g = fcpool.tile([128, N_CORES, 2, B], BF16, name=f"f1g{h}")
        for c8 in range(N_CORES):
            nc.scalar.dma_start(f1g[:, c8], agf1_dsts[h][c8])
        for c8 in range(N_CORES):
            for tt in range(2):
                gkt = c8 * 4 + 2 * h + tt
                nc.tensor.matmul(pf2b[:], f1g[:, c8, tt, :], w2sb2[:, gkt, :],
                                 start=(kt2 == 0), stop=False)
                kt2 += 1
    nc.tensor.matmul(pf2b[:], ones_bf[:], b2csb[:], start=False, stop=True)
    nc.scalar.activation(f2[:], pf2b[:], RELU)
    f2T = fcpool.tile([128, 4, B], BF16, name="f2T")
    for h in range(2):
        pt = psum.tile([128, 2, 64], BF16, name="pt", tag="pt", bufs=1)
        for i, t in enumerate((2 * h, 2 * h + 1)):
            nc.tensor.transpose(pt[:, i, :], f2[:, 128 * t:128 * (t + 1)],
                                identsb[:])
        nc.scalar.activation(f2T[:, 2 * h:2 * h + 2, :].rearrange(
            "p a b -> p (a b)"), pt[:].rearrange("p a b -> p (a b)"), COPY)
        nc.scalar.dma_start(agf2_srcs[h][:], f2T[:, 2 * h:2 * h + 2, :])
        nc.gpsimd.collective_compute(
            "AllGather", mybir.AluOpType.bypass,
            replica_groups=[list(range(N_CORES))],
            ins=[agf2_srcs[h][:].opt()], outs=[agf2_dsts[h][:].opt()])

    pacer(4, 2)

    # ========================================================================
    # fc3: [64, 125] slice per core; host concatenates the 8 slices
    # ========================================================================
    pf3 = psum.tile([64, 128], F32, name="pf3", tag="pf3", bufs=1)
    kt3 = 0
    for h in range(2):
        f2g = fcpool.tile([128, N_CORES, 2, B], BF16, name=f"f2g{h}")
        for c8 in range(N_CORES):
            nc.scalar.dma_start(f2g[:, c8], agf2_dsts[h][c8])
        for c8 in range(N_CORES):
            for tt in range(2):
                gkt = c8 * 4 + 2 * h + tt
                nc.tensor.matmul(pf3[:64, :125], f2g[:, c8, tt, :],
                                 w3fsb[:, gkt, :], start=(kt3 == 0), stop=False)
                kt3 += 1
    nc.tensor.matmul(pf3[:64, :125], ones_bf[:], b3fsb[:],
                     start=False, stop=True)
    osb = fcpool.tile([64, 125], F32, name="osb")
    nc.scalar.activation(osb[:], pf3[:64, :125], COPY)
    nc.scalar.dma_start(out[:], osb[:])

    w3p_cm.__exit__(None, None, None)
    w2p_cm.__exit__(None, None, None)
    fc_cm.__exit__(None, None, None)
    mid_cm.__exit__(None, None, None)
    w1_cm.__exit__(None, None, None)
    psum_cm.__exit__(None, None, None)
    const_cm.__exit__(None, None, None)
    tc_cm.__exit__(None, None, None)

    nc.compile()
    return nc


# ----------------------------------------------------------------------------
# host-side input preparation
# ----------------------------------------------------------------------------

def _rep_strips(xph, dt):
    """phase imgs [B,2,2,3,115,115] -> per-core replicated strips [96,111,W1S]."""
    reps = []
    for c in range(N_CORES):
        ph = xph[c * BL:(c + 1) * BL]  # [8, 2, 2, 3, 115, 115]
        rep = np.zeros((96, 111, W1S), dt)
        k = 0
        for g01 in range(2):
            for a in range(4):
                blk = ph[:, :, :, :, a:a + 111, :]  # [8,2,2,3,111,115]
                v = np.zeros((2, 2, 3, 111, BL, 115), dt)
                v[:, :, :, :, :, :115 - g01] = np.transpose(
                    blk[..., g01:], (1, 2, 3, 4, 0, 5))
                rep[k:k + 12, :, :920] = v.reshape(12, 111, BL * 115)
                k += 12
        reps.append(rep)
    return reps


def _prep_conv1(x):
    """x [B,3,224,224] fp32 -> per-core replicated tap strips [96,111,W1S]."""
    Bb = x.shape[0]
    xpad = np.zeros((Bb, 3, 230, 230), np.float32)
    xpad[:, :, 3:227, 3:227] = x
    xph = np.empty((Bb, 2, 2, 3, 115, 115), np.float32)
    for p in range(2):
        for q in range(2):
            xph[:, p, q] = xpad[:, :, p:p + 229:2, q:q + 229:2]
    return _rep_strips(xph.astype(ml_dtypes.bfloat16), ml_dtypes.bfloat16)


def _prep_w1(w1):
    """w1 [64,3,7,7] -> w1g [2 groups, 96, 64] fp32 (zero-padded taps)."""
    w1g = np.zeros((2, 96, 64), np.float32)
    for g in range(2):
        k = 0
        for g01 in range(2):
            for a in range(4):
                for p in range(2):
                    for q in range(2):
                        di = 2 * a + p
                        dj = 2 * (g01 + 2 * g) + q
                        for c in range(3):
                            if di <= 6 and dj <= 6:
                                w1g[g, k] = w1[:, c, di, dj]
                            k += 1
    return w1g


def _w1_tiles(w1flat):
    """[100, 128, 512] -> tile-major [13, 128, 8, 512] (last tile zero-pad)."""
    out = np.zeros((13, 128, 8, 512), w1flat.dtype)
    for g in range(13):
        nkt = min(8, 100 - 8 * g)
        out[g, :, :nkt, :] = w1flat[8 * g:8 * g + nkt].transpose(1, 0, 2)
    return out


def _spp_counts():
    cnt = np.empty(50, np.float32)
    for kk, (i0, i1, j0, j1) in enumerate(_spp_bins()):
        cnt[kk] = (i1 - i0) * (j1 - j0)
    return cnt


_CACHED = {}


def kernel(**inputs):
    if "nc" not in _CACHED:
        _CACHED["nc"] = build_program()
    nc = _CACHED["nc"]

    x = np.asarray(inputs["x"], np.float32)
    reps = _prep_conv1(x)
    w1gv = _prep_w1(np.asarray(inputs["w1"], np.float32)).astype(
        ml_dtypes.bfloat16)
    b1r = np.asarray(inputs["b1"], np.float32)
    b1v = np.concatenate([b1r, b1r]).reshape(128, 1)

    fc1_w = np.asarray(inputs["fc1_w"], np.float32)
    fc1_b = np.asarray(inputs["fc1_b"], np.float32)
    fc2_w = np.asarray(inputs["fc2_w"], np.float32)
    fc2_b = np.asarray(inputs["fc2_b"], np.float32)
    fc3_w = np.asarray(inputs["fc3_w"], np.float32)
    fc3_b = np.asarray(inputs["fc3_b"], np.float32)

    cnt = _spp_counts()
    w1s = fc1_w.reshape(4096, 256, 50) / cnt[None, None, :]
    # device feature d = (ct*50 + k)*128 + c128 -> channel ct*128+c128, bin k
    w1d = np.ascontiguousarray(
        w1s.reshape(4096, 2, 128, 50).transpose(1, 3, 2, 0))  # [2, 50, 128, 4096]

    w2cT = fc2_w.T  # [4096 in, 4096 out]
    w3Tv = fc3_w.T  # [4096 in, 1000 out]
    ident = np.eye(64, dtype=ml_dtypes.bfloat16)

    conv_w = {}
    for i in range(4):
        conv_w[f"w2_{i}"] = np.ascontiguousarray(
            np.asarray(inputs[f"w2_{i}"], np.float32).transpose(2, 3, 1, 0))
        conv_w[f"b2_{i}"] = np.asarray(inputs[f"b2_{i}"], np.float32).reshape(128, 1)
    for i in range(6):
        conv_w[f"w3_{i}"] = np.ascontiguousarray(
            np.asarray(inputs[f"w3_{i}"], np.float32).transpose(2, 3, 1, 0)
        ).astype(ml_dtypes.bfloat16)
        conv_w[f"b3_{i}"] = np.ascontiguousarray(
            np.asarray(inputs[f"b3_{i}"], np.float32).reshape(2, 128).T)

    in_maps = []
    for c in range(N_CORES):
        sl = slice(512 * c, 512 * (c + 1))
        sl3 = slice(125 * c, 125 * (c + 1))
        m = {
            "c1rep": reps[c],
            "w1g": w1gv,
            "b1": b1v,
            "w1c": _w1_tiles(np.ascontiguousarray(w1d[:, :, :, sl]).reshape(
                100, 128, 512).astype(ml_dtypes.bfloat16)),
            "b1c": fc1_b[sl].reshape(1, 512).astype(ml_dtypes.bfloat16),
            "w2c": np.ascontiguousarray(
                w2cT[:, sl].reshape(32, 128, 512).transpose(1, 0, 2)
            ).astype(ml_dtypes.bfloat16),
            "b2c": fc2_b[sl].reshape(1, 512).astype(ml_dtypes.bfloat16),
            "w3T": np.ascontiguousarray(
                w3Tv[:, sl3].reshape(32, 128, 125).transpose(1, 0, 2)
            ).astype(ml_dtypes.bfloat16),
            "b3f": fc3_b[sl3].reshape(1, 125).astype(ml_dtypes.bfloat16),
            "ident": ident,
        }
        m.update(conv_w)
        in_maps.append(m)

    res = run_bass_kernel_spmd(
        nc, in_maps, core_ids=list(range(N_CORES)),
        trace=bool(_CACHED.get("trace")), tmpdir=_CACHED.get("tmpdir"),
        trace_cores=_CACHED.get("trace_cores"))
    _CACHED["last_result"] = res
    return np.concatenate(
        [np.asarray(r["out"], np.float32) for r in res.results], axis=1)

